# revision 1
# baseline (speedup 1.0000x reference)
"""Trainium2 Bass kernel for BCNet-style bilinear head.

Computes logits[b,h,n,d] = sum_k hm[h,k] * v_[b,n,k] * q_[b,d,k] + h_bias
where v_ = v @ wn(Wv,gv).T + bv,  q_ = q @ wn(Wq,gq).T + bq,
wn(W,g) = (g/||W||_F) * W.

Head-parallel M-route (120 GF total vs 150 GF for the GT-route):
expand the product; per head h (= per core):
  Mt[c',c]   = sum_k hm[h,k]*Wq'[k,c'] * Wv'[k,c]     (C x C, batch-indep)
  PT[c,bd]   = sum_c' Mt[c',c] * qT[c',bd] (+ u[c])   (u absorbs bq-term)
  out[b,n,d] = sum_c vT[b,c,n] * PT[c,b*D+d] + t3[b,d]
  t3[bd]     = sum_c' w[c'] * qT[c',bd] + t4          (bv-term + const)
with u[c] = sum_k hm*bq*Wv'[k,c], w[c'] = sum_k hm*bv*Wq'[k,c'],
t4 = sum_k hm*bv*bq + h_bias[h].
Sharding: head-parallel over H=8 across 8 cores; each core consumes the
full v/q (replicated) and emits out[:, h] — no collectives.
All matmuls bf16 with fp32 PSUM accumulation.

PSUM: one pool, 8 tags of [128,512] (16KB/part), reused by tag across
M / t3 / P / out phases. vT tiles ride a 48-slot ring over the retired
wqh/wv SBUF slots (M pass 2 walks kt in reverse so high-kt slots retire
first). P is software-pipelined one batch ahead of out to hide the
PSUM->SBUF copy latency.
"""

import sys

for _p in ("/opt/trn_rl_repo",):
    if _p not in sys.path:
        sys.path.insert(0, _p)

import numpy as np
import ml_dtypes

from concourse import bass, bacc, tile, mybir
from concourse.bass_utils import run_bass_kernel_spmd

BF16 = ml_dtypes.bfloat16
F32 = mybir.dt.float32
BF = mybir.dt.bfloat16
AF = mybir.ActivationFunctionType

B, N, C, D, K, H = 16, 1024, 1024, 128, 3072, 8
KT, CT, NT = K // 128, C // 128, N // 128  # 24, 8, 8
BD = B * D  # 2048
NCORES = 8
XU, XW, XT4 = 0, CT, 2 * CT  # cst columns: u tiles, w tiles, t4

_CACHE = {}


def _build_program(repeat=1):
    nc = bacc.Bacc("TRN2", target_bir_lowering=False, debug=False,
                   num_devices=NCORES)

    # wqh[kt,p,c'] = hm[h,k]*Wq'[k,c'], k = kt*128+p   (per-core, head h)
    wqh_d = nc.dram_tensor("wqh", [KT, 128, C], BF, kind="ExternalInput")
    wv_d = nc.dram_tensor("wv", [KT, 128, C], BF, kind="ExternalInput")
    # qT[ct,p,b*128+d] = q[b,d,ct*128+p]
    qT_d = nc.dram_tensor("qT", [CT, 128, BD], BF, kind="ExternalInput")
    # vT[b,ct,p,n] = v[b,n,ct*128+p]
    vT_d = nc.dram_tensor("vT", [B, CT, 128, N], BF, kind="ExternalInput")
    cst_d = nc.dram_tensor("cst", [128, 2 * CT + 1], F32, kind="ExternalInput")
    one_d = nc.dram_tensor("one", [128, 1], BF, kind="ExternalInput")
    oner_d = nc.dram_tensor("oner", [1, 128], BF, kind="ExternalInput")
    out_d = nc.dram_tensor("out", [B, N, D], BF, kind="ExternalOutput")

    with tile.TileContext(nc) as tc:
        with (
            tc.tile_pool(name="wq", bufs=1) as p_wq,
            tc.tile_pool(name="wv", bufs=1) as p_wv,
            tc.tile_pool(name="qt", bufs=1) as p_qt,
            tc.tile_pool(name="mt", bufs=1) as p_mt,
            tc.tile_pool(name="pt", bufs=1) as p_pt,
            tc.tile_pool(name="t3", bufs=1) as p_t3,
            tc.tile_pool(name="small", bufs=1) as p_small,
            tc.tile_pool(name="ob", bufs=1) as p_ob,
            tc.tile_pool(name="vt", bufs=1) as p_vt,
            tc.tile_pool(name="ps", bufs=1, space="PSUM") as ps,
        ):
          for rep in range(repeat):
            R = f"r{rep}_"
            # ---- DMA: kt=0 weights first so M starts immediately ----
            # M pass 1 consumes wq[kt] + the low c-half of wv[kt] at
            # 1.7us/kt; streaming only those (1.09us/kt) keeps the PE fed.
            # wv high halves follow afterward (pass 2's window). kt=0's wq
            # is further split so the first matmul waits on a 128KB DMA.
            wq_sb, wv_sb = [], []
            for kt in range(KT):
                tv = [p_wv.tile([128, 512], BF, tag=f"wv{kt}{hb}",
                                name=f"{R}wv{kt}{hb}") for hb in "ab"]
                wv_sb.append(tv)
                if kt == 0:
                    tq = tuple(
                        p_wq.tile([128, 512], BF, tag=f"wq0{hb}",
                                  name=f"{R}wq0{hb}") for hb in "ab")
                    nc.sync.dma_start(tq[0][:], wqh_d.ap()[0, :, 0:512])
                    nc.scalar.dma_start(tv[0][:], wv_d.ap()[0, :, 0:512])
                    nc.scalar.dma_start(tq[1][:], wqh_d.ap()[0, :, 512:1024])
                    wq_sb.append(tq)
                else:
                    tq = p_wq.tile([128, C], BF, tag=f"wq{kt}",
                                   name=f"{R}wq{kt}")
                    nc.sync.dma_start(tq[:], wqh_d.ap()[kt])
                    wq_sb.append(tq)
                    nc.sync.dma_start(tv[0][:], wv_d.ap()[kt, :, 0:512])
                if kt == 0:
                    # small consts ride the scalar engine's DGE queue
                    cst_sb = p_small.tile([128, 2 * CT + 1], F32, tag="cst",
                                          name=f"{R}cst")
                    nc.scalar.dma_start(cst_sb[:], cst_d.ap())
                    one_sb = p_small.tile([128, 1], BF, tag="one",
                                          name=f"{R}one")
                    nc.scalar.dma_start(one_sb[:], one_d.ap())
                    oner_sb = p_small.tile([1, 128], BF, tag="oner",
                                           name=f"{R}oner")
                    nc.scalar.dma_start(oner_sb[:], oner_d.ap())
                    qt_sb = p_qt.tile([128, CT * BD], BF, tag="qt",
                                      name=f"{R}qt")

            # wv high halves (pass 2 runs kt reversed, so send them
            # reversed), then qT, then vT — all riding the bus window left
            # idle once the pass-1 stream ends
            for kt in range(KT - 1, -1, -1):
                nc.sync.dma_start(wv_sb[kt][1][:],
                                  wv_d.ap()[kt, :, 512:1024])
            for g in range(CT):
                nc.sync.dma_start(qt_sb[:, g * BD:(g + 1) * BD], qT_d.ap()[g])

            # ---- t3 partials on DVE (runs during M) ----
            ta = p_t3.tile([128, BD], BF, tag="ta", name=f"{R}ta")
            tb = p_t3.tile([128, BD], BF, tag="tb", name=f"{R}tb")
            nc.vector.tensor_scalar_mul(ta[:], qt_sb[:, 0:BD],
                                        cst_sb[:, XW:XW + 1])
            for ct in range(1, CT):
                nc.vector.tensor_scalar_mul(
                    tb[:], qt_sb[:, ct * BD:(ct + 1) * BD],
                    cst_sb[:, XW + ct:XW + ct + 1])
                nc.vector.tensor_tensor(ta[:], ta[:], tb[:],
                                        mybir.AluOpType.add)

            # ---- M: Mt[c',c] = sum_k wqh[k,c']*wv[k,c] ----
            # two c-half passes; pass 2 reversed so high-kt tiles retire
            # first (their SBUF slots become the vT ring, below)
            mt_sb = [p_mt.tile([128, C], BF, tag=f"mt{i}", name=f"{R}mt{i}")
                     for i in range(CT)]
            for half in range(2):
                kts = list(range(KT)) if half == 0 else \
                    list(range(KT - 1, -1, -1))
                pms = [ps.tile([128, 512], F32, tag=f"t{i}",
                               name=f"{R}pm{half}_{i}") for i in range(CT)]
                for kt in kts:
                    for i in range(CT):
                        if kt == 0:
                            lhsT = wq_sb[0][i // 4][:, (i % 4) * 128:
                                                    (i % 4 + 1) * 128]
                        else:
                            lhsT = wq_sb[kt][:, i * 128:(i + 1) * 128]
                        nc.tensor.matmul(
                            pms[i][:], lhsT, wv_sb[kt][half][:],
                            start=(kt == kts[0]), stop=(kt == kts[-1]))
                for i in range(CT):
                    dst = mt_sb[i][:, half * 512:(half + 1) * 512]
                    if i % 2 == 0:
                        nc.scalar.activation(dst, pms[i][:], AF.Copy)
                    else:
                        nc.vector.tensor_copy(dst, pms[i][:])

            # ---- t3 row: partition-reduce + t4. Issued on PE after P_0
            # (t3row is first needed by out_0's adds, which read it with a
            # partition-broadcast AP), so P_0 starts the moment M finishes.
            t3row = p_t3.tile([1, BD], BF, tag="t3row", name=f"{R}t3row")
            t3bc = p_t3.tile([128, BD], BF, tag="t3bc", name=f"{R}t3bc")

            def t3_phase():
                for j in range(4):
                    pt3 = ps.tile([1, 512], F32, tag=f"t{j}",
                                  name=f"{R}t3ps{j}")
                    nc.tensor.matmul(pt3[:], one_sb[:, 0:1],
                                     ta[:, j * 512:(j + 1) * 512],
                                     start=True, stop=True)
                    dst = t3row[0:1, j * 512:(j + 1) * 512]
                    if j % 2 == 0:
                        nc.scalar.activation(dst, pt3[:], AF.Identity,
                                             bias=cst_sb[0:1, XT4:XT4 + 1],
                                             scale=1.0)
                    else:
                        nc.vector.tensor_scalar_add(
                            dst, pt3[:], cst_sb[0:1, XT4:XT4 + 1])
                for j in range(4):
                    pb = ps.tile([128, 512], F32, tag=f"t{4 + j}",
                                 name=f"{R}t3bc{j}")
                    nc.tensor.matmul(pb[:], oner_sb[:],
                                     t3row[0:1, j * 512:(j + 1) * 512],
                                     start=True, stop=True)
                    if j % 2 == 0:
                        nc.scalar.activation(t3bc[:, j * 512:(j + 1) * 512],
                                             pb[:], AF.Copy)
                    else:
                        nc.vector.tensor_copy(
                            t3bc[:, j * 512:(j + 1) * 512], pb[:])

            # ---- per batch: P_b (pipelined one ahead) + out_{b-1} ----
            # pt is a 2-batch ring: out_b reads what P_b just wrote, so
            # only 2 slices per ct-tile need to live (frees 28KB/part of
            # SBUF for a much deeper vT prefetch ring)
            pt_sb = [p_pt.tile([128, 2 * 128], BF, tag=f"pt{i}",
                               name=f"{R}pt{i}") for i in range(CT)]
            vts = {}

            # vT ring: 7 dedicated fresh slots (usable before M retires
            # anything), the retired t3 scratch tiles (tb frees ~30us, ta
            # after the t3 reduce), then the 23 wq slots in pass-2 retire
            # order
            NVP = 19
            ring_slots = ([(p_vt, f"vtp{i}") for i in range(NVP)]
                          + [(p_t3, "tb"), (p_t3, "ta")]
                          + [(p_wq, f"wq{KT - 1 - i}") for i in range(KT - 1)])
            RING = len(ring_slots)

            def load_vt(b):
                vts[b] = []
                for ct in range(CT):
                    pool, tag = ring_slots[(b * CT + ct) % RING]
                    t = pool.tile([128, C], BF, tag=tag, name=f"{R}vt{b}_{ct}")
                    nc.sync.dma_start(t[:], vT_d.ap()[b, ct])
                    vts[b].append(t)

            def p_phase(b):
                for ct in range(CT):
                    pp = ps.tile([128, 128], F32, tag=f"t{ct}",
                                 name=f"{R}pp{b}_{ct}")
                    for j in range(CT):
                        nc.tensor.matmul(
                            pp[:],
                            mt_sb[j][:, ct * 128:(ct + 1) * 128],
                            qt_sb[:, j * BD + b * 128:j * BD + (b + 1) * 128],
                            start=(j == 0), stop=(j == CT - 1))
                    nc.scalar.activation(
                        pt_sb[ct][:, (b % 2) * 128:(b % 2 + 1) * 128], pp[:],
                        AF.Identity, bias=cst_sb[:, XU + ct:XU + ct + 1],
                        scale=1.0)

            def out_phase(b):
                ob = p_ob.tile([128, NT * D], BF, tag=f"ob{b % 2}",
                               name=f"{R}ob{b}")
                for nt in range(NT):
                    po = ps.tile([128, 128], F32, tag=f"t{nt}",
                                 name=f"{R}po{b}_{nt}")
                    for ct in range(CT):
                        nc.tensor.matmul(
                            po[:],
                            vts[b][ct][:, nt * 128:(nt + 1) * 128],
                            pt_sb[ct][:, (b % 2) * 128:(b % 2 + 1) * 128],
                            start=(ct == 0), stop=(ct == CT - 1))
                    nc.vector.tensor_tensor(
                        ob[:, nt * D:(nt + 1) * D], po[:],
                        t3bc[:, b * 128:(b + 1) * 128],
                        mybir.AluOpType.add)
                # one store per batch, on Activation's DGE queue (doesn't
                # block the sync-queue vT load stream); the final batch
                # stores per-nt on alternating queues so each fires right
                # after its own add (sub-range deps) and the tail shrinks
                if b == B - 1:
                    for g in range(4):
                        eng = nc.scalar if g % 2 == 0 else nc.sync
                        eng.dma_start(
                            out_d.ap()[b, g * 256:(g + 1) * 256, :]
                            .rearrange("(nt p) d -> p nt d", p=128),
                            ob[:, g * 2 * D:(g + 1) * 2 * D]
                            .rearrange("p (nt d) -> p nt d", nt=2))
                else:
                    nc.scalar.dma_start(
                        out_d.ap()[b].rearrange("(nt p) d -> p nt d", p=128),
                        ob[:].rearrange("p (nt d) -> p nt d", nt=NT))
                del vts[b]

            load_vt(0)
            p_phase(0)
            t3_phase()
            for b in range(1, B):
                load_vt(b)
                p_phase(b)
                out_phase(b - 1)
            out_phase(B - 1)

    nc.compile()
    return nc


def _get_program(repeat=1):
    key = f"nc{repeat}"
    if key not in _CACHE:
        _CACHE[key] = _build_program(repeat)
    return _CACHE[key]


def _prep_inputs(v, q, Wv, gv, bv, Wq, gq, bq, h_mat, h_bias):
    v = np.asarray(v, np.float32)
    q = np.asarray(q, np.float32)
    Wv = np.asarray(Wv, np.float32)
    Wq = np.asarray(Wq, np.float32)
    bv = np.asarray(bv, np.float32)
    bq = np.asarray(bq, np.float32)
    sv = np.float32(gv) / np.float32(np.linalg.norm(Wv))
    sq = np.float32(gq) / np.float32(np.linalg.norm(Wq))
    hm = np.asarray(h_mat, np.float32)[0, :, 0, :]  # (H, K)
    hb = np.asarray(h_bias, np.float32).reshape(H)

    Wvp = Wv * sv  # (K, C)
    Wqp = Wq * sq
    wv_b = np.ascontiguousarray(Wvp.reshape(KT, 128, C)).astype(BF16)
    qT = np.ascontiguousarray(
        q.transpose(2, 0, 1).reshape(CT, 128, BD)).astype(BF16)
    vT = np.ascontiguousarray(
        v.transpose(0, 2, 1).reshape(B, CT, 128, N)).astype(BF16)
    one = np.ones((128, 1), BF16)
    oner = np.ones((1, 128), BF16)

    in_maps = []
    for h in range(NCORES):
        wqh = np.ascontiguousarray(
            (hm[h][:, None] * Wqp).reshape(KT, 128, C)).astype(BF16)
        u = (hm[h] * bq) @ Wvp  # (C,)
        w = (hm[h] * bv) @ Wqp  # (C,)
        t4 = float((hm[h] * bv) @ bq) + float(hb[h])
        cst = np.zeros((128, 2 * CT + 1), np.float32)
        cst[:, XU:XU + CT] = u.reshape(CT, 128).T
        cst[:, XW:XW + CT] = w.reshape(CT, 128).T
        cst[0, XT4] = t4
        in_maps.append({
            "wqh": wqh,
            "wv": wv_b,
            "qT": qT,
            "vT": vT,
            "cst": cst,
            "one": one,
            "oner": oner,
        })
    return in_maps


def run_device(in_maps, **kw):
    nc = _get_program()
    return run_bass_kernel_spmd(nc, in_maps, list(range(NCORES)), **kw)


def kernel(v, q, Wv, gv, bv, Wq, gq, bq, h_mat, h_bias):
    in_maps = _prep_inputs(v, q, Wv, gv, bv, Wq, gq, bq, h_mat, h_bias)
    res = run_device(in_maps)
    out = np.empty((B, H, N, D), np.float32)
    for h in range(NCORES):
        out[:, h] = res.results[h]["out"].astype(np.float32)
    return out


if __name__ == "__main__":
    rng = np.random.default_rng(0)
    ins = {
        "v": rng.standard_normal((B, N, C), np.float32),
        "q": rng.standard_normal((B, D, C), np.float32),
        "Wv": rng.standard_normal((K, C), np.float32) * 0.02,
        "gv": np.ones((), np.float32),
        "bv": rng.standard_normal((K,), np.float32) * 0.02,
        "Wq": rng.standard_normal((K, C), np.float32) * 0.02,
        "gq": np.ones((), np.float32),
        "bq": rng.standard_normal((K,), np.float32) * 0.02,
        "h_mat": rng.standard_normal((1, H, 1, K), np.float32) * 0.02,
        "h_bias": np.zeros((1, H, 1, 1), np.float32),
    }
    out = kernel(**ins)
    print("out", out.shape, out.dtype, np.abs(out).max())



# revision 33
# speedup vs baseline: 1.3428x; 1.3428x over previous
"""Trainium2 Bass kernel for BCNet-style bilinear head — fp8 DoubleRow route.

Computes logits[b,h,n,d] = sum_k hm[h,k] * v_[b,n,k] * q_[b,d,k] + h_bias
where v_ = v @ wn(Wv,gv).T + bv,  q_ = q @ wn(Wq,gq).T + bq,
wn(W,g) = (g/||W||_F) * W.

Head-parallel M-route (one head per core, no collectives):
  M[c',c]  = sum_k wqh[k,c']*wv[k,c]      wqh = hm[h]*Wq', wv = Wv'
  P[c,bd]  = sum_c' M[c',c]*qT[c',bd] + u[c]
  outT[d,n]= sum_c vT[c,n]*P[c,b*D+d] + t3[d,b]
  t3[d,b]  = sum_c' w[c']*qT[c',b*D+d] + t4

All matmuls fp8e4m3 with DoubleRow perf mode (2 k-tiles per instruction,
0.5 cycles/row => 4x bf16 throughput in the cost model) and fp32 PSUM.
Precision via hi/lo splits (x = x_hi + x_lo, both e4m3 at one power-2
scale; the lo part rides subnormals):
  M stage: 3 terms (qh*vh + qh*vl + ql*vh)   ~bf16 accuracy
  P stage: 3 terms (Mt and qT both split)
  O stage: 2 terms (v plain e4m3, P split)   dominant error ~1.6e-2
Mt is stored at the scale that makes the P matmul's PSUM land directly in
P's storage scale (c2=1), so the P split is act(bias=u) + one
scalar_tensor_tensor. Out tiles are [d, n] so the t3 term is a per-
partition activation bias and the d-major store has 2KB DMA rows.

PSUM: 8 banks of [128,512] f32, one accumulation group each (the
interpreter enforces one live group per 2KB zero region). M runs as two
8-group waves (c'-tiles 0-3 / 4-7); wave B re-DMAs the wqh column halves
into the wave A tags. P uses banks 0-3, O banks 4-7, software-pipelined
one bd-slice (4 batches) apart. vT streams through a 3-slot ring.
"""

import sys

for _p in ("/opt/trn_rl_repo",):
    if _p not in sys.path:
        sys.path.insert(0, _p)

import math

import numpy as np
import ml_dtypes

from concourse import bass, bacc, tile, mybir
from concourse.bass_utils import run_bass_kernel_spmd

F8NP = ml_dtypes.float8_e4m3
BF16 = ml_dtypes.bfloat16
F32 = mybir.dt.float32
BF = mybir.dt.bfloat16
F8 = mybir.dt.float8e4
AF = mybir.ActivationFunctionType
DR = mybir.MatmulPerfMode.DoubleRow
ALU = mybir.AluOpType

B, N, C, D, K, H = 16, 1024, 1024, 128, 3072, 8
BD = B * D                      # 2048
KP, CP, CT = K // 256, C // 256, C // 128  # 12, 4, 8
NCORES = 8
XU, XT4 = 0, CT                 # cst columns

_CACHE = {}


def _pow2(x):
    return 2.0 ** round(math.log2(x))


def _build_program(c1, c3, ct3, debug=False):
    nc = bacc.Bacc("TRN2", target_bir_lowering=False, debug=False,
                   num_devices=NCORES)

    # DRAM layouts are partition-dim-outermost: the simulator DMA is a flat
    # element copy in AP order, so src/dst dims must iterate identically
    wqh_d = {s: nc.dram_tensor(f"wqh_{s}", [128, KP, 2, C], F8,
                               kind="ExternalInput") for s in ("h", "l")}
    wv_d = {s: nc.dram_tensor(f"wv_{s}", [128, KP, 2, C], F8,
                              kind="ExternalInput") for s in ("h", "l")}
    qt_d = {s: nc.dram_tensor(f"qt_{s}", [128, CP, 2, BD], F8,
                              kind="ExternalInput") for s in ("h", "l")}
    vt_d = nc.dram_tensor("vt", [B, 128, CP, 2, N], F8, kind="ExternalInput")
    wt_d = nc.dram_tensor("wt", [128, CP, 2, 2], F8, kind="ExternalInput")
    cst_d = nc.dram_tensor("cst", [128, CT + 1], F32, kind="ExternalInput")
    out_d = nc.dram_tensor("out", [B, 128, N], BF, kind="ExternalOutput")
    if debug:
        dbg_d = {
            "mt_h": nc.dram_tensor("dbg_mt_h", [128, CP, 2, C], F8,
                                   kind="ExternalOutput"),
            "mt_l": nc.dram_tensor("dbg_mt_l", [128, CP, 2, C], F8,
                                   kind="ExternalOutput"),
            "pt_h": nc.dram_tensor("dbg_pt_h", [128, CP, 2, BD], F8,
                                   kind="ExternalOutput"),
            "pt_l": nc.dram_tensor("dbg_pt_l", [128, CP, 2, BD], F8,
                                   kind="ExternalOutput"),
            "t3t": nc.dram_tensor("dbg_t3t", [128, B], F32,
                                  kind="ExternalOutput"),
            "wqbh": nc.dram_tensor("dbg_wqbh", [128, KP, 2, 512], F8,
                                   kind="ExternalOutput"),
            "wqah": nc.dram_tensor("dbg_wqah", [128, KP, 2, 512], F8,
                                   kind="ExternalOutput"),
            "wvh": nc.dram_tensor("dbg_wvh", [128, KP, 2, C], F8,
                                  kind="ExternalOutput"),
            "qth": nc.dram_tensor("dbg_qth", [128, CP, 2, BD], F8,
                                  kind="ExternalOutput"),
            "wqbl": nc.dram_tensor("dbg_wqbl", [128, KP, 2, 512], F8,
                                   kind="ExternalOutput"),
        }

    with tile.TileContext(nc) as tc:
        with (
            tc.tile_pool(name="wq", bufs=1) as p_wq,
            tc.tile_pool(name="wv", bufs=1) as p_wv,
            tc.tile_pool(name="qt", bufs=1) as p_qt,
            tc.tile_pool(name="mt", bufs=1) as p_mt,
            tc.tile_pool(name="pt", bufs=1) as p_pt,
            tc.tile_pool(name="vt", bufs=1) as p_vt,
            tc.tile_pool(name="small", bufs=1) as p_small,
            tc.tile_pool(name="ob", bufs=1) as p_ob,
            tc.tile_pool(name="ps", bufs=1, space="PSUM") as ps,
        ):
            # wave A consumes in stream order: hi*hi first, then lo*hi
            # (wq-lo is the lighter stream), then hi*lo
            TERMS = [("h", "h"), ("l", "h"), ("h", "l")]

            cst_sb = p_small.tile([128, CT + 1], F32, tag="cst", name="cst")
            wt_sb = p_small.tile([128, CP, 2, 2], F8, tag="wt", name="wt")

            # ---- wave A weight streams on sync queue (kp-interleaved) ----
            # strip tiles with chunked DMAs: few dma_starts (HWDGE is
            # ~0.64us/dma, shared) while sub-range deps feed the PE finely.
            # wqa/wqb: [128, KP, 2, 512] column halves for waves A/B.
            wqa = {s: p_wq.tile([128, KP, 2, 512], F8, tag=f"wqa{s}",
                                name=f"wqa{s}") for s in ("h", "l")}
            wqbh = p_wq.tile([128, KP, 2, 512], F8, tag="wqbh", name="wqbh")
            wv_t = {s: p_wv.tile([128, KP, 2, C], F8, tag=f"wv{s}",
                                 name=f"wv{s}") for s in ("h", "l")}

            def wq_chunk(dst, dname, r0, r1, c0=0, c1=512, cof=0):
                nc.sync.dma_start(dst[:, r0:r1, :, c0:c1],
                                  wqh_d[dname].ap()[:, r0:r1, :,
                                                    cof + c0:cof + c1])

            def wv_chunk(s, r0, r1, c0=0, c1=C):
                # wv rides the scalar queue: doubles the weight-stream issue
                # rate (sync SEQ holds ~1.2us per dma through HWDGE)
                nc.scalar.dma_start(wv_t[s][:, r0:r1, :, c0:c1],
                                    wv_d[s].ap()[:, r0:r1, :, c0:c1])

            # early chunks fine (latency ramp), later chunks coarse
            wq_chunk(wqa["h"], "h", 0, 1, 0, 128)
            wv_chunk("h", 0, 1, 0, 512)
            wq_chunk(wqa["h"], "h", 0, 1, 128, 512)
            wv_chunk("h", 0, 1, 512, 1024)
            for r0, r1 in ((1, 2), (2, 3), (3, 4), (4, 6), (6, 9), (9, 12)):
                wq_chunk(wqa["h"], "h", r0, r1)
                wv_chunk("h", r0, r1)
            for r0, r1 in ((0, 4), (4, 8), (8, 12)):   # wq-lo (2nd sweep)
                wq_chunk(wqa["l"], "l", r0, r1)
            for r0, r1 in ((0, 3), (3, 6), (6, 9), (9, 12)):  # wv-lo (3rd)
                wv_chunk("l", r0, r1)
            # wave B hi strip (own tile, no WAR): after the wave A streams
            for r0, r1 in ((0, 4), (4, 8), (8, 12)):
                nc.sync.dma_start(wqbh[:, r0:r1],
                                  wqh_d["h"].ap()[:, r0:r1, :, 512:1024])
            # small consts ride the scalar queue after the weight streams
            # (cst is first read by the P phase, wt by t3)
            nc.scalar.dma_start(cst_sb[:], cst_d.ap())
            nc.scalar.dma_start(wt_sb[:], wt_d.ap())
            wq_sb = {}
            for kp in range(KP):
                for s in ("h", "l"):
                    wq_sb[(kp, 0, s)] = wqa[s][:, kp]
                wq_sb[(kp, 1, "h")] = wqbh[:, kp]
            wv_sb = {(kp, s): wv_t[s][:, kp]
                     for kp in range(KP) for s in ("h", "l")}

            # qt tiles; DMA is issued on the sync queue AFTER the wave A
            # weight stream so the 4MB transfer doesn't squat on the DMA
            # engines while wave A is being fed (needed first by t3 ~88us)
            qt_sb = {}
            for s in ("h", "l"):
                t = p_qt.tile([128, CP, 2, BD], F8, tag=f"qt{s}", name=f"qt{s}")
                qt_sb[s] = t

            # ---- M: two waves of 8 full-bank groups [128,512] ----
            mt_sb = {s: p_mt.tile([128, CP, 2, C], F8, tag=f"mt{s}",
                                  name=f"mt{s}") for s in ("h", "l")}

            def m_split(wave, i, pm_i, pm_ch):
                ii = wave * 4 + i           # global c'-tile
                hid = mt_sb["h"][:, ii // 2, ii % 2,
                                 pm_ch * 512:(pm_ch + 1) * 512]
                lod = mt_sb["l"][:, ii // 2, ii % 2,
                                 pm_ch * 512:(pm_ch + 1) * 512]
                nc.scalar.activation(hid, pm_i[:], AF.Copy, scale=c1)
                nc.vector.scalar_tensor_tensor(lod, pm_i[:], c1, hid,
                                               ALU.mult, ALU.subtract)

            def m_wave_stream(wave):
                # term-outer: consumes the kp-ordered DMA streams; all 8
                # groups accumulate together and stop at wave end
                pms = {}
                for i in range(4):
                    for ch in range(2):
                        pms[(i, ch)] = ps.tile([128, 512], F32,
                                               tag=f"t{i * 2 + ch}",
                                               name=f"pm{wave}_{i}_{ch}")
                for ti, (sq, sv) in enumerate(TERMS):
                    for kp in range(KP):
                        for i in range(4):
                            lhsT = wq_sb[(kp, wave, sq)][:, :,
                                                         i * 128:(i + 1) * 128]
                            for ch in range(2):
                                nc.tensor.matmul(
                                    pms[(i, ch)][:],
                                    lhsT,
                                    wv_sb[(kp, sv)][:, :,
                                                    ch * 512:(ch + 1) * 512],
                                    start=(ti == 0 and kp == 0),
                                    stop=(ti == 2 and kp == KP - 1),
                                    perf_mode=DR)
                for i in range(4):
                    for ch in range(2):
                        m_split(wave, i, pms[(i, ch)], ch)

            # wave B consumes (h,l) before (l,h): wv-l is resident while
            # wqb-l is WAR-gated behind wave A's lo sweep
            TERMS_B = [("h", "h"), ("h", "l"), ("l", "h")]

            def m_wave_grouped(wave):
                # group-outer: weights already resident; each c'-tile's two
                # groups finish staggered so the splits spread out instead of
                # bursting at wave end (P(0) needs them all)
                for i in range(4):
                    pms = [ps.tile([128, 512], F32, tag=f"t{i * 2 + ch}",
                                   name=f"pm{wave}_{i}_{ch}")
                           for ch in range(2)]
                    for ti, (sq, sv) in enumerate(TERMS_B):
                        for kp in range(KP):
                            lhsT = wq_sb[(kp, wave, sq)][:, :,
                                                         i * 128:(i + 1) * 128]
                            for ch in range(2):
                                nc.tensor.matmul(
                                    pms[ch][:],
                                    lhsT,
                                    wv_sb[(kp, sv)][:, :,
                                                    ch * 512:(ch + 1) * 512],
                                    start=(ti == 0 and kp == 0),
                                    stop=(ti == 2 and kp == KP - 1),
                                    perf_mode=DR)
                    for ch in range(2):
                        m_split(wave, i, pms[ch], ch)

            if debug:
                nc.scalar.dma_start(dbg_d["wqah"].ap(), wqa["h"][:])
                nc.scalar.dma_start(dbg_d["wvh"].ap(), wv_t["h"][:])
            m_wave_stream(0)
            # wave B wq-lo re-tiles the wqa-l tag; emitted after wave A's
            # matmuls so the WAR dependency is in program order
            wqbl = p_wq.tile([128, KP, 2, 512], F8, tag="wqal", name="wqbl")
            for kp in range(KP):
                wq_sb[(kp, 1, "l")] = wqbl[:, kp]
            for r0, r1 in ((0, 4), (4, 8), (8, 12)):
                wq_chunk(wqbl, "l", r0, r1, cof=512)
            for s in ("h", "l"):
                nc.sync.dma_start(qt_sb[s][:], qt_d[s].ap())
            m_wave_grouped(1)

            # ---- t3T[d, b]: 16 sequential column groups in bank t0 ----
            t3ps = ps.tile([128, B], F32, tag="t0", name="t3ps")
            T3TERMS = [("h", 0), ("h", 1), ("l", 0)]  # (sq, w hi/lo)
            for b in range(B):
                first, last = (0, "h", 0), (CP - 1, "l", 0)
                for cp in range(CP):
                    for sq, jw in T3TERMS:
                        nc.tensor.matmul(
                            t3ps[:, b:b + 1],
                            qt_sb[sq][:, cp, :, b * D:b * D + 128],
                            wt_sb[:, cp, :, jw:jw + 1],
                            start=(cp, sq, jw) == first,
                            stop=(cp, sq, jw) == last,
                            perf_mode=DR)
            t3t = p_small.tile([128, B], F32, tag="t3t", name="t3t")
            nc.scalar.activation(t3t[:], t3ps[:], AF.Identity,
                                 bias=cst_sb[:, XT4:XT4 + 1], scale=ct3)

            # ---- P + O pipelined per bd-slice of 512 (4 batches) ----
            pt_sb = {s: p_pt.tile([128, CP, 2, BD], F8, tag=f"pt{s}",
                                  name=f"pt{s}") for s in ("h", "l")}
            vt_sb = {}

            NVT = 4

            def load_vt(b):
                t = p_vt.tile([128, CP, 2, N], F8, tag=f"vtp{b % NVT}",
                              name=f"vt{b}")
                nc.sync.dma_start(t[:], vt_d.ap()[b])
                vt_sb[b] = t

            def p_quarter(sl, qi):
                lo, hi_ = sl * 512, (sl + 1) * 512
                for ct in (2 * qi, 2 * qi + 1):
                    pp = ps.tile([128, 512], F32, tag=f"t{ct % 4}",
                                 name=f"pp{sl}_{ct}")
                    first, last = (0, "h", "h"), (CP - 1, "l", "h")
                    for cp in range(CP):
                        for sm, sq in TERMS:
                            nc.tensor.matmul(
                                pp[:],
                                mt_sb[sm][:, cp, :, ct * 128:(ct + 1) * 128],
                                qt_sb[sq][:, cp, :, lo:hi_],
                                start=(cp, sm, sq) == first,
                                stop=(cp, sm, sq) == last,
                                perf_mode=DR)
                    hid = pt_sb["h"][:, ct // 2, ct % 2, lo:hi_]
                    lod = pt_sb["l"][:, ct // 2, ct % 2, lo:hi_]
                    ucol = cst_sb[:, XU + ct:XU + ct + 1]
                    nc.scalar.activation(hid, pp[:], AF.Identity, bias=ucol,
                                         scale=1.0)
                    nc.vector.scalar_tensor_tensor(
                        lod, pp[:], ucol, hid, ALU.add, ALU.subtract)

            def o_batch(b):
                ob = p_ob.tile([128, N], BF, tag=f"ob{b % 3}", name=f"ob{b}")
                for nh in range(2):
                    po = ps.tile([128, 512], F32,
                                 tag=f"t{4 + (b % 2) * 2 + nh}",
                                 name=f"po{b}_{nh}")
                    first, last = ("h", 0), ("l", CP - 1)
                    for s in ("h", "l"):
                        for cp in range(CP):
                            nc.tensor.matmul(
                                po[:],
                                pt_sb[s][:, cp, :, b * D:b * D + 128],
                                vt_sb[b][:, cp, :, nh * 512:(nh + 1) * 512],
                                start=(s, cp) == first,
                                stop=(s, cp) == last,
                                perf_mode=DR)
                    nc.scalar.activation(ob[:, nh * 512:(nh + 1) * 512],
                                         po[:], AF.Identity,
                                         bias=t3t[:, b:b + 1], scale=c3)
                    if b == B - 1:
                        eng = nc.sync if nh == 0 else nc.scalar
                        eng.dma_start(
                            out_d.ap()[b, :, nh * 512:(nh + 1) * 512],
                            ob[:, nh * 512:(nh + 1) * 512])
                del vt_sb[b]
                if b != B - 1:
                    eng = nc.sync if b % 2 == 0 else nc.scalar
                    eng.dma_start(out_d.ap()[b], ob[:])

            # P(0) up front, then one quarter of P(b//4+1) between o-batches
            # so late vt transfers hide behind PE work
            if debug:
                nc.scalar.dma_start(dbg_d["wqbh"].ap(), wqbh[:])
                nc.scalar.dma_start(dbg_d["wqbl"].ap(), wqbl[:])
                nc.scalar.dma_start(dbg_d["qth"].ap(), qt_sb["h"][:])
                nc.scalar.dma_start(dbg_d["mt_h"].ap(), mt_sb["h"][:])
                nc.scalar.dma_start(dbg_d["mt_l"].ap(), mt_sb["l"][:])
                nc.scalar.dma_start(dbg_d["t3t"].ap(), t3t[:])
            for b_ in range(NVT - 1):
                load_vt(b_)
            for qi in range(4):
                p_quarter(0, qi)
            for b_ in range(B):
                nsl = b_ // 4 + 1
                if nsl < 4:
                    p_quarter(nsl, b_ % 4)
                o_batch(b_)
                if b_ + NVT - 1 < B:
                    load_vt(b_ + NVT - 1)

            if debug:
                nc.scalar.dma_start(dbg_d["pt_h"].ap(), pt_sb["h"][:])
                nc.scalar.dma_start(dbg_d["pt_l"].ap(), pt_sb["l"][:])

    nc.compile()
    return nc


def _get_program(c1, c3, ct3):
    key = (c1, c3, ct3)
    if key not in _CACHE:
        _CACHE[key] = _build_program(c1, c3, ct3)
    return _CACHE[key]


def _hilo(x, s):
    xs = np.clip(x * s, -192.0, 192.0)
    hi = xs.astype(F8NP)
    lo = np.clip(xs - hi.astype(np.float32), -192.0, 192.0).astype(F8NP)
    return hi, lo


def _prep_inputs(v, q, Wv, gv, bv, Wq, gq, bq, h_mat, h_bias):
    v = np.asarray(v, np.float32)
    q = np.asarray(q, np.float32)
    Wv = np.asarray(Wv, np.float32)
    Wq = np.asarray(Wq, np.float32)
    bv = np.asarray(bv, np.float32)
    bq = np.asarray(bq, np.float32)
    sv_ = np.float32(gv) / np.float32(np.linalg.norm(Wv))
    sq_ = np.float32(gq) / np.float32(np.linalg.norm(Wq))
    hm = np.asarray(h_mat, np.float32)[0, :, 0, :]  # (H, K)
    hb = np.asarray(h_bias, np.float32).reshape(H)

    Wvp = Wv * sv_  # (K, C)
    Wqp = Wq * sq_

    # power-2 scales; targets: inputs sigma*s ~ 1, M storage sigma ~ 0.25
    sg_wq = max(float(np.std(hm)) * float(np.std(Wqp)), 1e-30)
    sg_wv = max(float(np.std(Wvp)), 1e-30)
    s_wq = _pow2(1.0 / sg_wq)
    s_wv = _pow2(1.0 / sg_wv)
    sg_M = math.sqrt(K) * sg_wq * sg_wv
    s_M = _pow2(0.25 / sg_M)
    s_q = 1.0
    s_v = 1.0
    c1 = s_M / (s_wq * s_wv)
    c3 = 1.0 / (s_v * s_M * s_q)

    qT = q.transpose(2, 0, 1).reshape(CP, 2, 128, BD).transpose(2, 0, 1, 3)
    qt_h, qt_l = _hilo(np.ascontiguousarray(qT), s_q)
    vT = v.transpose(0, 2, 1).reshape(B, CP, 2, 128, N).transpose(0, 3, 1, 2, 4)
    vt8 = np.clip(np.ascontiguousarray(vT) * s_v, -192, 192).astype(F8NP)

    wv_p = np.ascontiguousarray(
        Wvp.reshape(KP, 2, 128, C).transpose(2, 0, 1, 3))
    wvh, wvl = _hilo(wv_p, s_wv)

    in_maps = []
    ct3 = 1.0
    for h in range(NCORES):
        wqh = (hm[h][:, None] * Wqp)
        wqh_p = np.ascontiguousarray(
            wqh.reshape(KP, 2, 128, C).transpose(2, 0, 1, 3))
        wqh_h, wqh_l = _hilo(wqh_p, s_wq)
        u = (hm[h] * bq) @ Wvp  # (C,)
        w = (hm[h] * bv) @ Wqp  # (C,)
        t4 = float((hm[h] * bv) @ bq) + float(hb[h])
        if h == 0:
            sg_w = max(float(np.std(w)), 1e-30)
            s_w = _pow2(1.0 / sg_w)
            ct3 = 1.0 / (s_w * s_q)
        wt = np.ascontiguousarray(
            w.reshape(CP, 2, 128).transpose(2, 0, 1))  # (128, CP, 2)
        wt_h, wt_l = _hilo(wt, s_w)
        wt8 = np.stack([wt_h, wt_l], axis=-1)  # (128, CP, 2, 2)
        cst = np.zeros((128, CT + 1), np.float32)
        cst[:, XU:XU + CT] = (u * (s_M * s_q)).reshape(CT, 128).T
        cst[:, XT4] = t4
        in_maps.append({
            "wqh_h": wqh_h, "wqh_l": wqh_l,
            "wv_h": wvh, "wv_l": wvl,
            "qt_h": qt_h, "qt_l": qt_l,
            "vt": vt8,
            "wt": wt8,
            "cst": cst,
        })
    return in_maps, (float(c1), float(c3), float(ct3))


def kernel(v, q, Wv, gv, bv, Wq, gq, bq, h_mat, h_bias):
    in_maps, scales = _prep_inputs(v, q, Wv, gv, bv, Wq, gq, bq, h_mat, h_bias)
    nc = _get_program(*scales)
    res = run_bass_kernel_spmd(nc, in_maps, list(range(NCORES)))
    out = np.empty((B, H, N, D), np.float32)
    for h in range(NCORES):
        # out_d is [B, 128(d), N]
        out[:, h] = res.results[h]["out"].astype(np.float32).transpose(0, 2, 1)
    return out


if __name__ == "__main__":
    rng = np.random.default_rng(0)
    ins = {
        "v": rng.standard_normal((B, N, C), np.float32),
        "q": rng.standard_normal((B, D, C), np.float32),
        "Wv": rng.standard_normal((K, C), np.float32) * 0.02,
        "gv": np.ones((), np.float32),
        "bv": rng.standard_normal((K,), np.float32) * 0.02,
        "Wq": rng.standard_normal((K, C), np.float32) * 0.02,
        "gq": np.ones((), np.float32),
        "bq": rng.standard_normal((K,), np.float32) * 0.02,
        "h_mat": rng.standard_normal((1, H, 1, K), np.float32) * 0.02,
        "h_bias": np.zeros((1, H, 1, 1), np.float32),
    }
    out = kernel(**ins)
    print("out", out.shape, out.dtype, np.abs(out).max())
